# revision 11
# baseline (speedup 1.0000x reference)
"""MeanAggregator (GNN message passing) Trainium2 Bass kernel.

Reference computation:
    neigh_idx = concat([neighbours, nodes[:, None]], axis=1)   # [B, K+1]
    out = features[neigh_idx].mean(axis=1)                     # [B, D]

Strategy: data-parallel over the 8 NeuronCores (12500 nodes each), feature
table replicated and pre-scaled by 1/11 on the host so the device only sums.

Per core the gather is done with the fast SWDGE `dma_gather` primitive
(int16 indices, so the 1M-row table is addressed through 31 buckets of 32768
rows; one gather call per (neighbour-slot j, bucket k) cell). Gathered rows
land in SBUF tiles in (j, bucket)-order; a `dma_scatter_add` per half-round
(one neighbour slot j, half the buckets) CCE-accumulates each row into a
parity-split pair of SBUF accumulators at the row's node slot. Rounds are
duplicate-free (one row per node per j), so concurrent CCE read-modify-write
races cannot occur within a call; across calls the Tile framework serializes
on the accumulator WAW hazard. Cell sizes are padded to the max across cores
(same SPMD program everywhere) with dummy rows (gather row 0 of the bucket,
scatter into a reserved dummy node slot).
"""

import numpy as np

B = 100000
K = 10
KP1 = K + 1
N = 1000000
D = 128
NCORES = 8
BPC = B // NCORES          # 12500 nodes per core
SBK = 32768                # bucket rows (int16 index range)
KSPLITS = 4                # scatter rounds per neighbour slot
P = 128


def _ceil_to(x, m):
    return (x + m - 1) // m * m


class Plan:
    """Static (shared-across-cores) program structure derived from max cell
    sizes. `cells` is a flat list in (j, k) order; `halves` groups them into
    scatter rounds."""

    def __init__(self, n_nodes, n_rows, bucket, cell_sizes, ksplits=2):
        # cell_sizes[j][k] = max over cores of #requests with slot j in bucket k
        self.n_nodes = n_nodes
        self.n_rows = n_rows
        self.bucket = bucket
        self.nbuck = (n_rows + bucket - 1) // bucket
        self.ng = max(1, -(-(n_nodes + 1) // 256))   # parity groups
        self.padn = self.ng * 256
        self.dummy = self.padn - 1
        assert self.padn > n_nodes
        kper = -(-self.nbuck // ksplits)
        self.halves = []      # list of dict(cells=[(k, Psize, gcol)], L, scol, j)
        gcol = 0
        scol = 0
        for j in range(KP1):
            for half in range(ksplits):
                ks = range(half * kper, min((half + 1) * kper, self.nbuck))
                cells = []
                L = 0
                for k in ks:
                    n = cell_sizes[j][k]
                    if n == 0:
                        continue
                    psz = _ceil_to(n, P)
                    cells.append((k, psz, gcol))
                    gcol += psz // 16
                    L += psz
                if L == 0:
                    continue
                self.halves.append(
                    dict(j=j, cells=cells, L=L, scol=scol)
                )
                scol += L // 16
        self.gcols = gcol
        self.scols = scol
        self.ch_max = max(h["L"] // P for h in self.halves)

    def signature(self):
        return (
            self.n_nodes,
            self.n_rows,
            self.bucket,
            tuple((h["j"], tuple(c[:2] for c in h["cells"])) for h in self.halves),
        )


def build_plan(idx_cores, n_nodes, n_rows, bucket, ksplits=2):
    """idx_cores: list of [n_nodes, KP1] int32 arrays (one per core)."""
    nbuck = (n_rows + bucket - 1) // bucket
    sizes = [[0] * nbuck for _ in range(KP1)]
    for idx in idx_cores:
        bk = idx // bucket          # [n_nodes, KP1]
        for j in range(KP1):
            cnt = np.bincount(bk[:, j], minlength=nbuck)
            for k in range(nbuck):
                if cnt[k] > sizes[j][k]:
                    sizes[j][k] = int(cnt[k])
    return Plan(n_nodes, n_rows, bucket, sizes, ksplits=ksplits)


def _wrap16(vals):
    """[L] int16 (L % 128 == 0) -> [128, L//16] SWDGE index layout
    (idx i at partition i%16, col i//16; replicated to all 8 Q7 groups)."""
    c = np.ascontiguousarray(vals.reshape(-1, 16).T)  # [16, L/16]
    return np.tile(c, (8, 1))


def prep_idx(idx_rows, plan):
    """Per-core gather/scatter index tensors for this plan."""
    n_nodes = idx_rows.shape[0]
    gidx = np.zeros((128, plan.gcols), np.int16)
    sidx = np.full((128, plan.scols), plan.dummy, np.int16)
    bk = idx_rows // plan.bucket
    lo = idx_rows % plan.bucket
    for h in plan.halves:
        j = h["j"]
        dst = np.full((h["L"],), plan.dummy, np.int16)
        off = 0
        for (k, psz, gcol) in h["cells"]:
            sel = np.nonzero(bk[:, j] == k)[0]
            n = sel.size
            assert n <= psz, (n, psz)
            g = np.zeros((psz,), np.int16)
            g[:n] = lo[sel, j]
            gidx[:, gcol : gcol + psz // 16] = _wrap16(g)
            dst[off : off + n] = sel
            off += psz
        sidx[:, h["scol"] : h["scol"] + h["L"] // 16] = _wrap16(dst)
    return {"gidx": gidx, "sidx": sidx}


def build_nc(plan, gather_bufs=2, num_queues=1):
    """Build + compile the per-core Bass program (SPMD: same NEFF everywhere)."""
    import concourse.bacc as bacc
    import concourse.mybir as mybir
    import concourse.tile as tile

    nc = bacc.Bacc(
        "TRN2",
        target_bir_lowering=False,
        debug=False,
        num_devices=NCORES,
        num_swdge_queues=num_queues,
    )
    feat = nc.dram_tensor(
        "features", [plan.n_rows, D], mybir.dt.float32, kind="ExternalInput"
    )
    gidx = nc.dram_tensor(
        "gidx", [128, plan.gcols], mybir.dt.int16, kind="ExternalInput"
    )
    sidx = nc.dram_tensor(
        "sidx", [128, plan.scols], mybir.dt.int16, kind="ExternalInput"
    )
    out = nc.dram_tensor(
        "out", [2 * 128, plan.ng * D], mybir.dt.float32, kind="ExternalOutput"
    )

    with tile.TileContext(nc) as tc:
        with (
            tc.tile_pool(name="idxp", bufs=1) as idxp,
            tc.tile_pool(name="gp", bufs=gather_bufs) as gp,
            tc.tile_pool(name="accp", bufs=1) as accp,
        ):
            gidx_sb = idxp.tile([128, plan.gcols], mybir.dt.int16)
            sidx_sb = idxp.tile([128, plan.scols], mybir.dt.int16)
            nc.sync.dma_start(out=gidx_sb[:], in_=gidx.ap())
            nc.sync.dma_start(out=sidx_sb[:], in_=sidx.ap())
            acc0 = accp.tile([128, plan.ng * D], mybir.dt.float32)
            acc1 = accp.tile([128, plan.ng * D], mybir.dt.float32)
            nc.vector.memset(acc0[:], 0.0)
            nc.vector.memset(acc1[:], 0.0)
            pool_dma_i = 0
            for hi, h in enumerate(plan.halves):
                t = gp.tile([128, plan.ch_max, D], mybir.dt.float32,
                            name=f"t{hi}", tag="t")
                off = 0
                for (k, psz, gcol) in h["cells"]:
                    kend = min(plan.n_rows, (k + 1) * plan.bucket)
                    nc.gpsimd.dma_gather(
                        out_ap=t[:, off : off + psz // P, :],
                        in_ap=feat.ap()[k * plan.bucket : kend, :],
                        idxs_ap=gidx_sb[:, gcol : gcol + psz // 16],
                        num_idxs=psz,
                        num_idxs_reg=psz,
                        elem_size=D,
                        queue_num=pool_dma_i % num_queues,
                    )
                    pool_dma_i += 1
                    off += psz // P
                nc.gpsimd.dma_scatter_add(
                    out_ap=acc0[:],
                    in_ap=t[:, 0 : h["L"] // P, :],
                    idxs_ap=sidx_sb[:, h["scol"] : h["scol"] + h["L"] // 16],
                    num_idxs=h["L"],
                    num_idxs_reg=h["L"],
                    elem_size=D,
                    sbuf_tokens_per_rank=128,
                    parity_reg=0,
                    out_ap_other=acc1[:],
                    single_packet=False,
                    queue_num=pool_dma_i % num_queues,
                )
                pool_dma_i += 1
            nc.sync.dma_start(out=out.ap()[0:128, :], in_=acc0[:])
            nc.sync.dma_start(out=out.ap()[128:256, :], in_=acc1[:])

    # SWDGE DMA-completion sem lanes are assigned round-robin (mod 8) over
    # Pool-engine DMAs in *scheduled* order, and each lane is locked to a
    # single queue. Tile reorders instructions at context exit, so rewrite
    # queue_num here, in final order: queue = i % num_queues (divides 8)
    # keeps every lane single-queue.
    import concourse.mybir as mb

    i = 0
    for blk in nc.m.functions[0].blocks:
        for inst in blk.instructions:
            if getattr(inst, "engine", None) == mybir.EngineType.Pool and isinstance(
                inst, (mb.InstDMAGatherAnt, mb.InstDMAScatterAddAnt)
            ):
                inst.queue_num = i % num_queues
                i += 1
    nc.compile()
    return nc


def decode_out(out_arr, plan):
    """[256, ng*D] accumulator dump -> [n_nodes, D] node-major."""
    r = out_arr.reshape(2, 128, plan.ng, D)
    return r.transpose(2, 0, 1, 3).reshape(plan.padn, D)[: plan.n_nodes]


_CACHE = {}


def kernel(nodes, neighbours, features):
    from concourse.bass_utils import run_bass_kernel_spmd

    nodes = np.asarray(nodes)
    neighbours = np.asarray(neighbours)
    features = np.asarray(features, dtype=np.float32)

    idx_all = np.empty((B, KP1), np.int32)
    idx_all[:, :K] = neighbours
    idx_all[:, K] = nodes
    idx_cores = [idx_all[c * BPC : (c + 1) * BPC] for c in range(NCORES)]

    plan = build_plan(idx_cores, BPC, N, SBK, ksplits=KSPLITS)
    sig = plan.signature()
    if _CACHE.get("sig") != sig:
        _CACHE["nc"] = build_nc(plan)
        _CACHE["sig"] = sig
        _CACHE["plan"] = plan
    nc = _CACHE["nc"]
    plan = _CACHE["plan"]

    fscaled = np.ascontiguousarray(features * np.float32(1.0 / KP1))
    in_maps = []
    for c in range(NCORES):
        m = {"features": fscaled}
        m.update(prep_idx(idx_cores[c], plan))
        in_maps.append(m)
    res = run_bass_kernel_spmd(nc, in_maps, core_ids=list(range(NCORES)))
    return np.concatenate(
        [decode_out(res.results[c]["out"], plan) for c in range(NCORES)], axis=0
    )


# revision 12
# speedup vs baseline: 1.0507x; 1.0507x over previous
"""MeanAggregator (GNN message passing) Trainium2 Bass kernel.

Reference computation:
    neigh_idx = concat([neighbours, nodes[:, None]], axis=1)   # [B, K+1]
    out = features[neigh_idx].mean(axis=1)                     # [B, D]

Strategy: data-parallel over the 8 NeuronCores (12500 nodes each), feature
table replicated and pre-scaled by 1/11 on the host so the device only sums.

Per core the gather is done with the fast SWDGE `dma_gather` primitive
(int16 indices, so the 1M-row table is addressed through 31 buckets of 32768
rows; one gather call per (neighbour-slot j, bucket k) cell). Gathered rows
land in SBUF tiles in (j, bucket)-order; a `dma_scatter_add` per half-round
(one neighbour slot j, half the buckets) CCE-accumulates each row into a
parity-split pair of SBUF accumulators at the row's node slot. Rounds are
duplicate-free (one row per node per j), so concurrent CCE read-modify-write
races cannot occur within a call; across calls the Tile framework serializes
on the accumulator WAW hazard. Cell sizes are padded to the max across cores
(same SPMD program everywhere) with dummy rows (gather row 0 of the bucket,
scatter into a reserved dummy node slot).
"""

import numpy as np

B = 100000
K = 10
KP1 = K + 1
N = 1000000
D = 128
NCORES = 8
BPC = B // NCORES          # 12500 nodes per core
SBK = 32768                # bucket rows (int16 index range)
KSPLITS = 4                # scatter rounds per neighbour slot
P = 128


def _ceil_to(x, m):
    return (x + m - 1) // m * m


class Plan:
    """Static (shared-across-cores) program structure derived from max cell
    sizes. `cells` is a flat list in (j, k) order; `halves` groups them into
    scatter rounds."""

    def __init__(self, n_nodes, n_rows, bucket, cell_sizes, ksplits=2):
        # cell_sizes[j][k] = max over cores of #requests with slot j in bucket k
        self.n_nodes = n_nodes
        self.n_rows = n_rows
        self.bucket = bucket
        self.nbuck = (n_rows + bucket - 1) // bucket
        self.ng = max(1, -(-(n_nodes + 1) // 256))   # parity groups
        self.padn = self.ng * 256
        self.dummy = self.padn - 1
        assert self.padn > n_nodes
        kper = -(-self.nbuck // ksplits)
        self.halves = []      # list of dict(cells=[(k, Psize, gcol)], L, scol, j)
        gcol = 0
        scol = 0
        for j in range(KP1):
            for half in range(ksplits):
                ks = range(half * kper, min((half + 1) * kper, self.nbuck))
                cells = []
                L = 0
                for k in ks:
                    n = cell_sizes[j][k]
                    if n == 0:
                        continue
                    psz = _ceil_to(n, P)
                    cells.append((k, psz, gcol))
                    gcol += psz // 16
                    L += psz
                if L == 0:
                    continue
                self.halves.append(
                    dict(j=j, cells=cells, L=L, scol=scol)
                )
                scol += L // 16
        self.gcols = gcol
        self.scols = scol
        self.ch_max = max(h["L"] // P for h in self.halves)

    def signature(self):
        return (
            self.n_nodes,
            self.n_rows,
            self.bucket,
            tuple((h["j"], tuple(c[:2] for c in h["cells"])) for h in self.halves),
        )


def build_plan(idx_cores, n_nodes, n_rows, bucket, ksplits=2):
    """idx_cores: list of [n_nodes, KP1] int32 arrays (one per core)."""
    nbuck = (n_rows + bucket - 1) // bucket
    sizes = [[0] * nbuck for _ in range(KP1)]
    for idx in idx_cores:
        bk = idx // bucket          # [n_nodes, KP1]
        for j in range(KP1):
            cnt = np.bincount(bk[:, j], minlength=nbuck)
            for k in range(nbuck):
                if cnt[k] > sizes[j][k]:
                    sizes[j][k] = int(cnt[k])
    return Plan(n_nodes, n_rows, bucket, sizes, ksplits=ksplits)


def _wrap16(vals):
    """[L] int16 (L % 128 == 0) -> [128, L//16] SWDGE index layout
    (idx i at partition i%16, col i//16; replicated to all 8 Q7 groups)."""
    c = np.ascontiguousarray(vals.reshape(-1, 16).T)  # [16, L/16]
    return np.tile(c, (8, 1))


def prep_idx(idx_rows, plan):
    """Per-core gather/scatter index tensors for this plan."""
    n_nodes = idx_rows.shape[0]
    gidx = np.zeros((128, plan.gcols), np.int16)
    sidx = np.full((128, plan.scols), plan.dummy, np.int16)
    bk = idx_rows // plan.bucket
    lo = idx_rows % plan.bucket
    for h in plan.halves:
        j = h["j"]
        dst = np.full((h["L"],), plan.dummy, np.int16)
        off = 0
        for (k, psz, gcol) in h["cells"]:
            sel = np.nonzero(bk[:, j] == k)[0]
            n = sel.size
            assert n <= psz, (n, psz)
            g = np.zeros((psz,), np.int16)
            g[:n] = lo[sel, j]
            gidx[:, gcol : gcol + psz // 16] = _wrap16(g)
            dst[off : off + n] = sel
            off += psz
        sidx[:, h["scol"] : h["scol"] + h["L"] // 16] = _wrap16(dst)
    return {"gidx": gidx, "sidx": sidx}


def build_nc(plan, gather_bufs=2, num_queues=4):
    """Build + compile the per-core Bass program (SPMD: same NEFF everywhere)."""
    import concourse.bacc as bacc
    import concourse.mybir as mybir
    import concourse.tile as tile

    nc = bacc.Bacc(
        "TRN2",
        target_bir_lowering=False,
        debug=False,
        num_devices=NCORES,
        num_swdge_queues=num_queues,
    )
    feat = nc.dram_tensor(
        "features", [plan.n_rows, D], mybir.dt.float32, kind="ExternalInput"
    )
    gidx = nc.dram_tensor(
        "gidx", [128, plan.gcols], mybir.dt.int16, kind="ExternalInput"
    )
    sidx = nc.dram_tensor(
        "sidx", [128, plan.scols], mybir.dt.int16, kind="ExternalInput"
    )
    out = nc.dram_tensor(
        "out", [2 * 128, plan.ng * D], mybir.dt.float32, kind="ExternalOutput"
    )

    with tile.TileContext(nc) as tc:
        with (
            tc.tile_pool(name="idxp", bufs=1) as idxp,
            tc.tile_pool(name="gp", bufs=gather_bufs) as gp,
            tc.tile_pool(name="accp", bufs=1) as accp,
        ):
            gidx_sb = idxp.tile([128, plan.gcols], mybir.dt.int16)
            sidx_sb = idxp.tile([128, plan.scols], mybir.dt.int16)
            nc.sync.dma_start(out=gidx_sb[:], in_=gidx.ap())
            nc.sync.dma_start(out=sidx_sb[:], in_=sidx.ap())
            acc0 = accp.tile([128, plan.ng * D], mybir.dt.float32)
            acc1 = accp.tile([128, plan.ng * D], mybir.dt.float32)
            nc.vector.memset(acc0[:], 0.0)
            nc.vector.memset(acc1[:], 0.0)
            pool_dma_i = 0
            for hi, h in enumerate(plan.halves):
                t = gp.tile([128, plan.ch_max, D], mybir.dt.float32,
                            name=f"t{hi}", tag="t")
                off = 0
                for (k, psz, gcol) in h["cells"]:
                    kend = min(plan.n_rows, (k + 1) * plan.bucket)
                    nc.gpsimd.dma_gather(
                        out_ap=t[:, off : off + psz // P, :],
                        in_ap=feat.ap()[k * plan.bucket : kend, :],
                        idxs_ap=gidx_sb[:, gcol : gcol + psz // 16],
                        num_idxs=psz,
                        num_idxs_reg=psz,
                        elem_size=D,
                        queue_num=pool_dma_i % num_queues,
                    )
                    pool_dma_i += 1
                    off += psz // P
                nc.gpsimd.dma_scatter_add(
                    out_ap=acc0[:],
                    in_ap=t[:, 0 : h["L"] // P, :],
                    idxs_ap=sidx_sb[:, h["scol"] : h["scol"] + h["L"] // 16],
                    num_idxs=h["L"],
                    num_idxs_reg=h["L"],
                    elem_size=D,
                    sbuf_tokens_per_rank=128,
                    parity_reg=0,
                    out_ap_other=acc1[:],
                    single_packet=False,
                    queue_num=pool_dma_i % num_queues,
                )
                pool_dma_i += 1
            nc.sync.dma_start(out=out.ap()[0:128, :], in_=acc0[:])
            nc.sync.dma_start(out=out.ap()[128:256, :], in_=acc1[:])

    # SWDGE DMA-completion sem lanes are assigned round-robin (mod 8) over
    # Pool-engine DMAs in *scheduled* order, and each lane is locked to a
    # single queue. Tile reorders instructions at context exit, so rewrite
    # queue_num here, in final order: queue = i % num_queues (divides 8)
    # keeps every lane single-queue.
    import concourse.mybir as mb

    i = 0
    for blk in nc.m.functions[0].blocks:
        for inst in blk.instructions:
            if getattr(inst, "engine", None) == mybir.EngineType.Pool and isinstance(
                inst, (mb.InstDMAGatherAnt, mb.InstDMAScatterAddAnt)
            ):
                inst.queue_num = i % num_queues
                i += 1
    nc.compile()
    return nc


def decode_out(out_arr, plan):
    """[256, ng*D] accumulator dump -> [n_nodes, D] node-major."""
    r = out_arr.reshape(2, 128, plan.ng, D)
    return r.transpose(2, 0, 1, 3).reshape(plan.padn, D)[: plan.n_nodes]


_CACHE = {}


def kernel(nodes, neighbours, features):
    from concourse.bass_utils import run_bass_kernel_spmd

    nodes = np.asarray(nodes)
    neighbours = np.asarray(neighbours)
    features = np.asarray(features, dtype=np.float32)

    idx_all = np.empty((B, KP1), np.int32)
    idx_all[:, :K] = neighbours
    idx_all[:, K] = nodes
    idx_cores = [idx_all[c * BPC : (c + 1) * BPC] for c in range(NCORES)]

    plan = build_plan(idx_cores, BPC, N, SBK, ksplits=KSPLITS)
    sig = plan.signature()
    if _CACHE.get("sig") != sig:
        _CACHE["nc"] = build_nc(plan)
        _CACHE["sig"] = sig
        _CACHE["plan"] = plan
    nc = _CACHE["nc"]
    plan = _CACHE["plan"]

    fscaled = np.ascontiguousarray(features * np.float32(1.0 / KP1))
    in_maps = []
    for c in range(NCORES):
        m = {"features": fscaled}
        m.update(prep_idx(idx_cores[c], plan))
        in_maps.append(m)
    res = run_bass_kernel_spmd(nc, in_maps, core_ids=list(range(NCORES)))
    return np.concatenate(
        [decode_out(res.results[c]["out"], plan) for c in range(NCORES)], axis=0
    )
